# revision 54
# baseline (speedup 1.0000x reference)
"""CCPL loss kernel for Trainium2, 8 NeuronCores, SPMD data-parallel over (batch, S-half).

Self-contained: takes the full unsharded inputs (as produced by the reference
setup_inputs), shards across 8 cores, runs one Bass/Tile program per core,
and reduces the per-core partial sums on the host.

Device program is a pure matmul pipeline: the neighbor gather and the
center-minus-neighbor difference d = f[:, cid] - f[:, nid] are computed on the
host (channel-major, fp16).  Per layer the device does: DMA d -> 3-layer MLP
(f16 matmuls, PSUM drained by ACT/DVE with fused bias+relu) -> y (yneg|yk) ->
NCE G'' = yneg^T yk = -G/tau, per 1024-col block: DVE min -> ACT exp+accum.
Per-block (min, expsum) partials go to DRAM; the host combines them into the
exact log-sum-exp (invariant to the bias), so only overflow safety matters.
MLP of layer i+1 is interleaved into NCE of layer i to keep PE fed while
ACT/DVE pace through the exps/reductions.
"""
import sys
import numpy as np

sys.path.insert(0, "/opt/trn_rl_repo")

from contextlib import ExitStack

import concourse.bass as bass
import concourse.tile as tile
from concourse import bacc, mybir, bass_utils

F32 = mybir.dt.float32
F32R = mybir.dt.float32r
F16 = mybir.dt.float16
AF = mybir.ActivationFunctionType
ALU = mybir.AluOpType

B = 4
NUM_S = 4096            # neighbor pairs per layer (S)
HALF = 2048             # q rows per core
TAU = 0.01
INVTAU = 100.0
LAYERS = [(64, 256 * 256), (128, 128 * 128), (256, 64 * 64), (512, 32 * 32)]  # (C, HW)
DBLK = 512              # d-columns per MLP block
KBLOCKS = [(0, 1024), (1024, 1024), (2048, 1024), (3072, 1024)]  # NCE k-blocks per strip
NQT = 4
KBW = 1024              # max k-block width (psum tile)
NBLOCKS = 12            # 4 q blocks + 8 k blocks (own half first)


def build_bass(layers=(0, 1, 2, 3), do_nce=True, do_mlp=True, do_lp=True, nstrips=16):
    nc = bacc.Bacc("TRN2", target_bir_lowering=False, debug=False)

    # ---- DRAM tensors ----
    dq, dka, dkb = {}, {}, {}
    w0t, w1t, w2t, b0d, b1d, b2d = {}, {}, {}, {}, {}, {}
    o_negm, o_ssum, o_lp = {}, {}, {}
    for i, (C, HW) in enumerate(LAYERS):
        Cout = C // 4
        dq[i] = nc.dram_tensor(f"dq{i}", [C, HALF], F16, kind="ExternalInput").ap()
        dka[i] = nc.dram_tensor(f"dka{i}", [C, HALF], F16, kind="ExternalInput").ap()
        dkb[i] = nc.dram_tensor(f"dkb{i}", [C, HALF], F16, kind="ExternalInput").ap()
        # packed weights [w0.T | w1.T | w2.T] and biases [b0 | b1 | b2(padded)]
        w0t[i] = nc.dram_tensor(f"wp{i}", [C, 2 * C + Cout], F16, kind="ExternalInput").ap()
        b0d[i] = nc.dram_tensor(f"bp{i}", [C, 4], F32, kind="ExternalInput").ap()
        o_negm[i] = nc.dram_tensor(f"negm{i}", [128, 64], F32, kind="ExternalOutput").ap()
        o_ssum[i] = nc.dram_tensor(f"ssum{i}", [128, 64], F32, kind="ExternalOutput").ap()
        o_lp[i] = nc.dram_tensor(f"lp{i}", [Cout, 1], F32, kind="ExternalOutput").ap()

    with tile.TileContext(nc) as tc, ExitStack() as ctx:
        wpool = ctx.enter_context(tc.tile_pool(name="w", bufs=1))
        dpool = ctx.enter_context(tc.tile_pool(name="d16", bufs=2))
        xpool = ctx.enter_context(tc.tile_pool(name="x", bufs=2))
        ypool = ctx.enter_context(tc.tile_pool(name="y", bufs=3))
        obuf = ctx.enter_context(tc.tile_pool(name="obuf", bufs=2))
        tinyp = ctx.enter_context(tc.tile_pool(name="tiny", bufs=4))
        nscp = ctx.enter_context(tc.tile_pool(name="nsc", bufs=2))
        epool = ctx.enter_context(tc.tile_pool(name="ex", bufs=3))
        spool = ctx.enter_context(tc.tile_pool(name="sc", bufs=3))

        def pool_sum_tree(src_f16, width, dst_col):
            # Pool-engine pairwise add tree over SBUF (Pool may not touch PSUM).
            s = spool.tile([128, KBW], F32, tag="sc")
            h = width // 2
            nc.gpsimd.tensor_add(s[:, 0:h], src_f16[:, 0:h], src_f16[:, h:2 * h])
            off = 0
            while h > 1:
                nh = h // 2
                nc.gpsimd.tensor_add(s[:, off + h:off + h + nh],
                                     s[:, off:off + nh], s[:, off + nh:off + h])
                off += h
                h = nh
            nc.gpsimd.tensor_copy(dst_col, s[:, off:off + 1])
        mpsum = ctx.enter_context(tc.tile_pool(name="mps", bufs=2, space="PSUM"))
        npsum = ctx.enter_context(tc.tile_pool(name="nps", bufs=3, space="PSUM"))

        # ---- weight / bias loaders (emitted per-layer inside the schedule) ----
        wsb = {}
        bsb = {}

        def emit_weights(i):
            C, HW = LAYERS[i]
            Cout = C // 4
            CB = (C + 127) // 128
            cw = min(128, C)
            WCOLS = 2 * C + Cout
            wt = wpool.tile([128, CB * WCOLS], F16, tag=f"wp{i}")
            nc.sync.dma_start(
                wt[:cw, :].rearrange("p (cb c) -> p cb c", cb=CB),
                w0t[i][:].rearrange("(cb p) c -> p cb c", p=cw))
            for j, off, cols in ((0, 0, C), (1, C, C), (2, 2 * C, Cout)):
                wsb[(i, j)] = [wt[:, cbi * WCOLS + off: cbi * WCOLS + off + cols]
                               for cbi in range(CB)]
            bt = wpool.tile([128, CB * 4], F32, tag=f"bp{i}")
            nc.sync.dma_start(
                bt[:cw, :].rearrange("p (cb c) -> p cb c", cb=CB),
                b0d[i][:].rearrange("(cb p) c -> p cb c", p=cw))
            bt3 = bt[:].rearrange("p (cb t) -> p cb t", t=4)
            for j in range(4):
                bsb[(i, j)] = bt3[:, :, j:j + 1]

        # ---- PSUM drain dispatcher: only DVE/ACT may read PSUM; ----
        # ---- split ~60% ACT / 40% DVE to balance both engines      ----
        drain_ctr = [0]

        def drain_relu(dst, ps_ap, bias_ap):
            drain_ctr[0] += 1
            if drain_ctr[0] % 3 != 1:
                nc.scalar.activation(dst, ps_ap, AF.Relu, bias=bias_ap, scale=1.0)
            else:
                nc.vector.tensor_scalar(dst, ps_ap, bias_ap, 0.0,
                                        op0=ALU.add, op1=ALU.max)

        def drain_y(dst, ps_ap, i, Cout, neg):
            drain_ctr[0] += 1
            if drain_ctr[0] % 3 != 1:
                if neg:
                    nc.scalar.activation(dst, ps_ap, AF.Identity,
                                         bias=bsb[(i, 3)][:Cout, 0, :], scale=-INVTAU)
                else:
                    nc.scalar.activation(dst, ps_ap, AF.Identity,
                                         bias=bsb[(i, 2)][:Cout, 0, :], scale=1.0)
            else:
                if neg:
                    nc.vector.tensor_scalar(dst, ps_ap, bsb[(i, 2)][:Cout, 0, :],
                                            -INVTAU, op0=ALU.add, op1=ALU.mult)
                else:
                    nc.vector.tensor_scalar(dst, ps_ap, bsb[(i, 2)][:Cout, 0, :],
                                            None, op0=ALU.add)

        # ---- per-layer emission closures (software-pipelined issue order) ----
        def emit_dma(i):
            C, HW = LAYERS[i]
            CB = (C + 127) // 128
            cw = min(128, C)
            tiles = []
            for j, src in enumerate((dq[i], dka[i], dkb[i])):
                t = dpool.tile([128, CB * HALF], F16, tag=f"d16_{j}")
                # one DMA per tensor: DRAM rows (cb*128+p) -> partition p, stripe cb
                nc.sync.dma_start(
                    t[:cw, :].rearrange("p (cb c) -> p cb c", cb=CB),
                    src[:].rearrange("(cb p) c -> p cb c", p=cw))
                tiles.append(t)
            # y layout: cols [0:2048) = yneg (q MLP out scaled by -INVTAU),
            #           cols [2048:6144) = yk (own half first)
            y = ypool.tile([128, NBLOCKS * DBLK], F16, tag="y")
            return tiles, y

        def emit_mlp_block(i, g, d16, y):
            C, HW = LAYERS[i]
            Cout = C // 4
            CB = (C + 127) // 128
            # MLP: x1 = relu(W0 d + b0); x2 = relu(W1 x1 + b1); y = W2 x2 + b2
            # stage 0 reads the f16 d tile directly (no cast stage)
            dt = d16[g // 4]
            c0 = (g % 4) * DBLK

            def src0(cbi):
                return dt[:, cbi * HALF + c0: cbi * HALF + c0 + DBLK]
            xsrc = src0
            for j in range(2):
                xout = xpool.tile([128, CB * DBLK], F16, tag="x")
                wt = wsb[(i, j)]
                bt = bsb[(i, j)]
                for cbo in range(CB):
                    cwo = min(128, C - cbo * 128)
                    ps = mpsum.tile([128, DBLK], F32, tag="mps")
                    for cbi in range(CB):
                        cwi = min(128, C - cbi * 128)
                        nc.tensor.matmul(
                            ps[:cwo, :],
                            wt[cbi][:cwi, cbo * 128: cbo * 128 + cwo],
                            xsrc(cbi)[:cwi, :],
                            start=(cbi == 0), stop=(cbi == CB - 1))
                    dst = xout[:cwo, cbo * DBLK:(cbo + 1) * DBLK]
                    drain_relu(dst, ps[:cwo, :], bt[:cwo, cbo, :])
                xsrc = (lambda xo: lambda cbi: xo[:, cbi * DBLK:(cbi + 1) * DBLK])(xout)
            # final linear -> y block
            ps = mpsum.tile([128, DBLK], F32, tag="mps")
            wt = wsb[(i, 2)]
            for cbi in range(CB):
                cwi = min(128, C - cbi * 128)
                nc.tensor.matmul(ps[:Cout, :], wt[cbi][:cwi, :Cout],
                                 xsrc(cbi)[:cwi, :],
                                 start=(cbi == 0), stop=(cbi == CB - 1))
            ydst = y[:Cout, g * DBLK:(g + 1) * DBLK]
            b2ap = bsb[(i, 2)][:Cout, 0, :]
            drain_y(ydst, ps[:Cout, :], i, Cout, neg=(g < 4))

        def emit_lp(i, y):
            C, HW = LAYERS[i]
            Cout = C // 4
            lp = obuf.tile([128, 1], F32, tag="lp")
            lp_scr = xpool.tile([128, HALF], F32, tag="x")
            nc.gpsimd.tensor_mul(lp_scr[:Cout, :], y[:Cout, 0:HALF],
                                 y[:Cout, HALF:2 * HALF])
            lpr = tinyp.tile([128, 1], F32, tag="lpr")
            s = spool.tile([128, 2048], F32, tag="lpt")
            h = 1024
            nc.gpsimd.tensor_add(s[:Cout, 0:h], lp_scr[:Cout, 0:h],
                                 lp_scr[:Cout, h:2 * h])
            off = 0
            while h > 1:
                nh = h // 2
                nc.gpsimd.tensor_add(s[:Cout, off + h:off + h + nh],
                                     s[:Cout, off:off + nh],
                                     s[:Cout, off + nh:off + h])
                off += h
                h = nh
            # undo the -INVTAU scale baked into yneg
            nc.gpsimd.tensor_scalar(lp[:Cout, :], s[:Cout, off:off + 1],
                                    -1.0 / INVTAU, None, op0=ALU.mult)
            nc.sync.dma_start(o_lp[i], lp[:Cout, :])

        def emit_nce_block(i, m, qt, y, mq, sq):
            # G'' = yneg^T yk = -INVTAU*G; per-block min + exp-sum (host combines)
            C, HW = LAYERS[i]
            Cout = C // 4
            yk_off = HALF
            k0, kw = KBLOCKS[qt]
            lhs = y[:Cout, m * 128:(m + 1) * 128]
            ps = npsum.tile([128, KBW], F32, tag="nps")
            for nn in range(kw // 512):
                nc.tensor.matmul(
                    ps[:, nn * 512:(nn + 1) * 512], lhs,
                    y[:Cout, yk_off + k0 + nn * 512: yk_off + k0 + (nn + 1) * 512],
                    start=True, stop=True)
            col = m * NQT + qt
            nc.vector.tensor_reduce(mq[:, col:col + 1], ps[:, :kw],
                                    axis=mybir.AxisListType.X, op=ALU.min)
            ex = epool.tile([128, KBW], F16, tag="ex")
            nc.scalar.activation(ex[:, :kw], ps[:, :kw], AF.Exp,
                                 bias=mq[:, col:col + 1], scale=-1.0)
            pool_sum_tree(ex, kw, sq[:, col:col + 1])

        # ---- static software-pipelined schedule ----
        # NCE-i emission window overlaps MLP of later layers so PE stays fed
        # while ACT paces through the exps.  PE budget per NCE window ~= ACT
        # window (76us) - NCE matmuls (27us): MLP l1+l2 fit in window 0; the
        # heavy MLP l3 is split across windows 1 and 2.
        def nce_units(i, y):
            C, HW = LAYERS[i]
            Cout = C // 4
            mq = nscp.tile([128, NQT * nstrips], F32, tag="mq")
            sq = nscp.tile([128, NQT * nstrips], F32, tag="sq")
            units = []
            for m in range(nstrips):
                for qt in range(NQT):
                    units.append(lambda m=m, qt=qt: emit_nce_block(i, m, qt, y, mq, sq))

            def out():
                nc.sync.dma_start(o_negm[i][:, :NQT * nstrips], mq[:, :NQT * nstrips])
                nc.sync.dma_start(o_ssum[i][:, :NQT * nstrips], sq[:, :NQT * nstrips])
            units.append(out)
            return units

        def interleave(nce, inserts):
            """Emit all nce closures; inserts = [(frac, closure)] fired when
            that fraction of the nce list has been emitted."""
            ins = sorted(inserts, key=lambda t: t[0])
            k = 0
            for bi, u in enumerate(nce):
                while k < len(ins) and ins[k][0] <= bi / max(1, len(nce)):
                    ins[k][1]()
                    k += 1
                u()
            while k < len(ins):
                ins[k][1]()
                k += 1

        full = (0, 1, 2, 3)
        if layers != full or not (do_mlp and do_nce):
            # simple fallback ordering for debug configs
            pend = []
            for i in range(4):
                if i not in layers:
                    continue
                emit_weights(i)
                d16, y = emit_dma(i)
                if do_mlp:
                    for g in range(NBLOCKS):
                        emit_mlp_block(i, g, d16, y)
                for u in pend:
                    u()
                pend = []
                if do_mlp and do_nce:
                    if do_lp:
                        emit_lp(i, y)
                    pend = nce_units(i, y)
            for u in pend:
                u()
        else:
            d16_0, y0 = emit_dma(0)
            emit_weights(0)
            for g in range(NBLOCKS):
                emit_mlp_block(0, g, d16_0, y0)
            emit_weights(1)
            d16_1, y1 = emit_dma(1)
            emit_lp(0, y0)
            nce0 = nce_units(0, y0)
            st = {}

            def mlp_closures(i):
                def pre():
                    emit_weights(i)
                    st[i] = emit_dma(i)
                blocks = [lambda g=g, i=i: emit_mlp_block(i, g, *st[i])
                          for g in range(NBLOCKS)]
                return pre, blocks

            pre2, mlp2 = mlp_closures(2)
            pre3, mlp3 = mlp_closures(3)
            ins0 = [(0.05 + 0.28 * g / 12, lambda g=g: emit_mlp_block(1, g, d16_1, y1))
                    for g in range(NBLOCKS)]
            ins0.append((0.36, pre2))
            ins0 += [(0.40 + 0.55 * g / 12, mlp2[g]) for g in range(NBLOCKS)]
            interleave(nce0, ins0)

            emit_lp(1, y1)
            nce1 = nce_units(1, y1)
            ins1 = [(0.02, pre3)]
            ins1 += [(0.08 + 0.88 * g / 6, mlp3[g]) for g in range(6)]
            interleave(nce1, ins1)

            emit_lp(2, st[2][1])
            nce2 = nce_units(2, st[2][1])
            ins2 = [(0.05 + 0.88 * (g - 6) / 6, mlp3[g]) for g in range(6, 12)]
            interleave(nce2, ins2)

            emit_lp(3, st[3][1])
            for u in nce_units(3, st[3][1]):
                u()

    nc.compile()
    return nc


def prep_in_maps(inputs):
    inp = {k: np.asarray(v) for k, v in inputs.items()}
    shared = {}
    for i, (C, HW) in enumerate(LAYERS):
        cid = inp[f"cid{i}"].astype(np.intp)
        nid = inp[f"nid{i}"].astype(np.intp)
        for b in range(B):
            for nm, key in (("q", f"fq{i}"), ("k", f"fk{i}")):
                f = np.ascontiguousarray(inp[key][b]).reshape(C, HW)
                d = np.take(f, cid, axis=1)
                d -= np.take(f, nid, axis=1)
                shared[(nm, i, b)] = d.astype(np.float16)
        Cout = C // 4
        shared[("wp", i)] = np.ascontiguousarray(np.concatenate(
            [inp[f"w{i}_0"].T, inp[f"w{i}_1"].T, inp[f"w{i}_2"].T],
            axis=1).astype(np.float16))
        bp = np.zeros((C, 4), np.float32)
        bp[:, 0] = inp[f"b{i}_0"]
        bp[:, 1] = inp[f"b{i}_1"]
        bp[:Cout, 2] = inp[f"b{i}_2"]
        bp[:Cout, 3] = -INVTAU * inp[f"b{i}_2"].astype(np.float64)
        shared[("bp", i)] = bp

    in_maps = []
    for core in range(8):
        b, h = core // 2, core % 2
        im = {}
        for i, (C, HW) in enumerate(LAYERS):
            dq = shared[("q", i, b)]
            dk = shared[("k", i, b)]
            im[f"dq{i}"] = dq[:, h * HALF:(h + 1) * HALF]
            im[f"dka{i}"] = dk[:, h * HALF:(h + 1) * HALF]
            im[f"dkb{i}"] = dk[:, (1 - h) * HALF:(2 - h) * HALF]
            im[f"wp{i}"] = shared[("wp", i)]
            im[f"bp{i}"] = shared[("bp", i)]
        in_maps.append(im)
    return in_maps


def host_reduce(results):
    tot = np.float64(0.0)
    for r in results:
        for i, (C, HW) in enumerate(LAYERS):
            # per-qt-block partials: negm[p, 4m+qt] = -INVTAU*max_G(block),
            # ssum[p, 4m+qt] = sum exp(negm - G'') over the block
            negm4 = r[f"negm{i}"].astype(np.float64).reshape(128, 16, NQT)
            sq4 = r[f"ssum{i}"].astype(np.float64).reshape(128, 16, NQT)
            lp = r[f"lp{i}"].astype(np.float64)
            b = negm4.min(axis=2)
            ssum = (sq4 * np.exp(b[:, :, None] - negm4)).sum(axis=2)
            lse = np.log(ssum) - b
            tot += lse.sum() - INVTAU * lp.sum()
    return np.float32(tot / (B * NUM_S))


_NC_CACHE = {}


def _get_nc():
    if "nc" not in _NC_CACHE:
        _NC_CACHE["nc"] = build_bass()
    return _NC_CACHE["nc"]


def kernel(**inputs):
    nc = _get_nc()
    in_maps = prep_in_maps(inputs)
    res = bass_utils.run_bass_kernel_spmd(nc, in_maps, core_ids=list(range(8)))
    return host_reduce(res.results)


if __name__ == "__main__":
    pass
